# revision 2
# baseline (speedup 1.0000x reference)
# Trainium2 Bass kernel for windowed cross-attention (nn_CrossAttention).
#
# Reference computation (shapes hardcoded):
#   B=4, N=1024 (=32x32), C=512, NH=8 heads, HD=64
#   q = x_l @ Wq + bq    -> [B, NH, N, HD]   (query patch = whole image)
#   k = x_s @ Wk + bk    -> [B, NH, N, HD]   patchified 4x4 -> 64 patches x 16 tok
#   v = x_s @ Wv + bv
#   out[b,h,P,l,:] = softmax(q[b,h,l,:] @ k_patch[b,h,P].T * HD**-0.5) @ v_patch[b,h,P]
#   out shape [4, 8, 64, 1024, 64] fp32  (512 MB -> memory-bound on output writes)
#
# Sharding: 8 cores = (B=4) x (head-half=2). Each core computes its b and 4 heads:
# per-core output [4, 64, 1024, 64] = 64 MB.
#
# Per-core dataflow (all on-chip after the input loads):
#   - load x_l[b], x_s[b]; transpose via PE to get C-major layouts; the x_s
#     transpose copies also permute tokens into patch-major order so K and V
#     come out patch-ordered for free.
#   - QT/KT = W.T @ x.T  (outC on partitions, tokens on free), fp32
#   - V = x.T.T @ Wv (tokens on partitions) with a ones-row matmul adding bv
#   - scores S = QT.T-slice @ KT (fp32), exp via ScalarE (scale folded in, no
#     max-subtraction: logits are O(1) by construction), segmented sums +
#     reciprocal + broadcast-normalize on VectorE
#   - P^T chunks via PE transpose; O = P^T.T @ blockdiag(V patches) where the
#     block-diagonal [128, 8*64] matrix zeroes cross-patch terms; this emits
#     outputs in natural [l, hd] layout, 8 patches per matmul.
#   - PSUM -> SBUF staging copies on ScalarE, 2 MB HWDGE DMAs to HBM.
#
# The P/V side of the second matmul runs in bf16 (fp32 accumulation in PSUM);
# projections, scores and softmax stay fp32. Measured vs float64 reference this
# gives ~4.6e-3 scale-relative absmax error (fp32 everywhere: ~7e-7, but fp32
# matmul is 4 cycles/row on the PE which would make the kernel PE-bound).

import numpy as np

B, N, C = 4, 1024, 512
NH, HD = 8, 64
H4 = 4          # heads per core
NPATCH = 64     # 4x4 key/value patches
PTOK = 16       # tokens per patch
SCALE = HD ** -0.5

_CACHE = {}


def _build_program(pv_bf16=True):
    import concourse.bass as bass
    import concourse.mybir as mybir
    from concourse import bacc
    from concourse.tile import TileContext

    f32 = mybir.dt.float32
    pv_dt = mybir.dt.bfloat16 if pv_bf16 else mybir.dt.float32
    X = mybir.AxisListType.X
    MULT = mybir.AluOpType.mult
    Exp = mybir.ActivationFunctionType.Exp
    Ident = mybir.ActivationFunctionType.Identity

    nc = bacc.Bacc("TRN2", target_bir_lowering=False, debug=False, num_devices=1)

    xl_d = nc.dram_tensor("xl", [N, C], f32, kind="ExternalInput")
    xs_d = nc.dram_tensor("xs", [N, C], f32, kind="ExternalInput")
    wq_d = nc.dram_tensor("wq", [C, 256], f32, kind="ExternalInput")
    wk_d = nc.dram_tensor("wk", [C, 256], f32, kind="ExternalInput")
    wv_d = nc.dram_tensor("wv", [C, 256], f32, kind="ExternalInput")
    bq_d = nc.dram_tensor("bq2", [128, 2], f32, kind="ExternalInput")
    bk_d = nc.dram_tensor("bk2", [128, 2], f32, kind="ExternalInput")
    bv_d = nc.dram_tensor("bv_row", [1, 256], f32, kind="ExternalInput")
    idf_d = nc.dram_tensor("ident_f32", [128, 128], f32, kind="ExternalInput")
    idp_d = nc.dram_tensor("ident_pv", [128, 128], pv_dt, kind="ExternalInput")
    msk_d = nc.dram_tensor("maskbd", [128, 512], pv_dt, kind="ExternalInput")
    one_d = nc.dram_tensor("ones_row", [1, 128], f32, kind="ExternalInput")
    out_d = nc.dram_tensor("out_c", [H4, NPATCH, N, HD], f32, kind="ExternalOutput")

    with TileContext(nc) as tc:
        with (
            tc.tile_pool(name="singles", bufs=1) as sg,
            tc.tile_pool(name="xnat", bufs=1) as xnat_p,
            tc.tile_pool(name="bdv", bufs=10) as bdv_p,
            tc.tile_pool(name="work", bufs=2) as wk_p,
            tc.tile_pool(name="pts", bufs=4) as pts_p,
            tc.tile_pool(name="small", bufs=4) as sm_p,
            tc.tile_pool(name="stage", bufs=2) as st_p,
            tc.tile_pool(name="psA", bufs=1, space="PSUM") as psA,
            tc.tile_pool(name="psB", bufs=2, space="PSUM") as psB,
            tc.tile_pool(name="psC", bufs=2, space="PSUM") as psC,
        ):
            # ---- constants ----
            wq = sg.tile([128, 4, 256], f32, name="wq_t")
            wk = sg.tile([128, 4, 256], f32, name="wk_t")
            wv = sg.tile([128, 4, 256], f32, name="wv_t")
            nc.sync.dma_start(wq[:], wq_d.ap().rearrange("(ko ki) m -> ki ko m", ki=128))
            nc.sync.dma_start(wk[:], wk_d.ap().rearrange("(ko ki) m -> ki ko m", ki=128))
            nc.sync.dma_start(wv[:], wv_d.ap().rearrange("(ko ki) m -> ki ko m", ki=128))
            bq2 = sg.tile([128, 2], f32, name="bq2_t")
            bk2 = sg.tile([128, 2], f32, name="bk2_t")
            bvr = sg.tile([1, 256], f32, name="bvr_t")
            nc.sync.dma_start(bq2[:], bq_d.ap())
            nc.sync.dma_start(bk2[:], bk_d.ap())
            nc.sync.dma_start(bvr[:], bv_d.ap())
            idf = sg.tile([128, 128], f32, name="idf_t")
            idp = sg.tile([128, 128], pv_dt, name="idp_t")
            msk = sg.tile([128, 512], pv_dt, name="msk_t")
            ones = sg.tile([1, 128], f32, name="ones_t")
            nc.sync.dma_start(idf[:], idf_d.ap())
            nc.sync.dma_start(idp[:], idp_d.ap())
            nc.sync.dma_start(msk[:], msk_d.ap())
            nc.sync.dma_start(ones[:], one_d.ap())

            # ---- persistent intermediates ----
            xlT = sg.tile([128, 4, N], f32, name="xlT")   # [c_lo, ko, token]
            xsT = sg.tile([128, 4, N], f32, name="xsT")   # tokens patch-permuted
            QT = sg.tile([128, 2, N], f32, name="QT")     # [outC_lo, outC_tile, token]
            KT = sg.tile([128, 2, N], f32, name="KT")     # tokens patch-permuted
            vperm = sg.tile([128, 8, 256], pv_dt, name="vperm")  # [tok(perm), tile, outC]

            # ---- x transposes (PE) ----
            # x_l: plain; x_s: token-permute into patch-major order during copy.
            for which, src_d, dstT, permute in ((0, xl_d, xlT, False), (1, xs_d, xsT, True)):
                xn = xnat_p.tile([128, 8, C], f32, tag="xnat")
                nc.sync.dma_start(xn[:], src_d.ap().rearrange("(tt p) c -> p tt c", p=128))
                for tt in range(8):
                    for ko in range(4):
                        tp = psB.tile([128, 128], f32, tag="pt_psum")
                        nc.tensor.transpose(tp[:], xn[:, tt, ko * 128:(ko + 1) * 128], idf[:])
                        if not permute:
                            nc.vector.tensor_copy(dstT[:, ko, tt * 128:(tt + 1) * 128], tp[:])
                        else:
                            blk = dstT[:, ko, tt * 128:(tt + 1) * 128]
                            blk3 = blk.rearrange("p (px k) -> p px k", px=8)
                            for dy in range(4):
                                src3 = tp[:, dy * 32:(dy + 1) * 32].rearrange(
                                    "p (px dx) -> p px dx", px=8)
                                nc.vector.tensor_copy(blk3[:, :, dy * 4:dy * 4 + 4], src3)

            # ---- projections ----
            # QT/KT: [outC, token] = W.T @ x.T ; bias per-partition via ScalarE.
            for wt, xt, dst, bias in ((wq, xlT, QT, bq2), (wk, xsT, KT, bk2)):
                for m in range(2):
                    pp = psC.tile([128, 1024], f32, tag="o_psum")
                    for n in range(2):
                        for ko in range(4):
                            nc.tensor.matmul(
                                pp[:, n * 512:(n + 1) * 512],
                                wt[:, ko, m * 128:(m + 1) * 128],
                                xt[:, ko, n * 512:(n + 1) * 512],
                                start=(ko == 0), stop=(ko == 3))
                    nc.scalar.activation(dst[:, m, :], pp[:], Ident,
                                         bias=bias[:, m:m + 1], scale=1.0)
            # V: [token(perm), outC] = x.T.T @ Wv + ones.T @ bv_row
            for tt in range(8):
                vp = psC.tile([128, 1024], f32, tag="o_psum")
                for ko in range(4):
                    nc.tensor.matmul(vp[:, :256], xsT[:, ko, tt * 128:(tt + 1) * 128],
                                     wv[:, ko, :], start=(ko == 0), stop=False)
                nc.tensor.matmul(vp[:, :256], ones[:, :],
                                 bvr[:], start=False, stop=True)
                nc.vector.tensor_copy(vperm[:, tt, :], vp[:, :256])

            # ---- blockdiag(V) per (head, group): bd[r, px*64+hd] =
            #      (r//16 == px) * V_perm[r, g, h*64+hd]  via mask multiply ----
            bd = {}
            for h in range(H4):
                for g in range(8):
                    t = bdv_p.tile([128, 512], pv_dt, tag="bdv")
                    nc.vector.tensor_tensor(
                        t.rearrange("p (px hd) -> p px hd", px=8),
                        msk.rearrange("p (px hd) -> p px hd", px=8),
                        vperm[:, g, h * 64:(h + 1) * 64][:, None, :].to_broadcast(
                            (128, 8, 64)),
                        MULT)
                    bd[(h, g)] = t

            # ---- main attention loop ----
            for h in range(H4):
                th, po = h // 2, (h % 2) * 64
                for qt in range(8):
                    sp = psA.tile([128, 1024], f32, tag="s_psum")
                    for n in range(2):
                        nc.tensor.matmul(
                            sp[:, n * 512:(n + 1) * 512],
                            QT[po:po + 64, th, qt * 128:(qt + 1) * 128],
                            KT[po:po + 64, th, n * 512:(n + 1) * 512],
                            start=True, stop=True)
                    E = wk_p.tile([128, 1024], f32, tag="E")
                    nc.scalar.activation(E[:], sp[:], Exp, scale=SCALE)
                    sums = sm_p.tile([128, 64], f32, tag="sums")
                    nc.vector.reduce_sum(sums[:], E.rearrange("p (g s) -> p g s", s=16),
                                         axis=X)
                    rcp = sm_p.tile([128, 64], f32, tag="rcp")
                    nc.vector.reciprocal(rcp[:], sums[:])
                    Pn = wk_p.tile([128, 1024], pv_dt, tag="Pn")
                    nc.vector.tensor_tensor(
                        Pn.rearrange("p (g s) -> p g s", s=16),
                        E.rearrange("p (g s) -> p g s", s=16),
                        rcp[:, :, None].to_broadcast((128, 64, 16)),
                        MULT)
                    stage = st_p.tile([128, 4096], f32, tag="stage")
                    for gp in range(4):
                        op = psC.tile([128, 1024], f32, tag="o_psum")
                        for j in range(2):
                            g = gp * 2 + j
                            ptp = psB.tile([128, 128], pv_dt, tag="pt_psum")
                            nc.tensor.transpose(ptp[:], Pn[:, g * 128:(g + 1) * 128],
                                                idp[:])
                            pts = pts_p.tile([128, 128], pv_dt, tag="pts")
                            nc.vector.tensor_copy(pts[:], ptp[:])
                            nc.tensor.matmul(op[:, j * 512:(j + 1) * 512], pts[:],
                                             bd[(h, g)], start=True, stop=True)
                        nc.scalar.copy(stage[:, gp * 1024:(gp + 1) * 1024], op[:])
                    dst = out_d.ap()[h][:, qt * 128:(qt + 1) * 128, :].rearrange(
                        "P l hd -> l P hd")
                    nc.sync.dma_start(dst, stage.rearrange("p (P hd) -> p P hd", hd=64))

    nc.compile()
    return nc


def _host_inputs(x_l, x_s, Wq, bq, Wk, bk, Wv, bv, pv_bf16=True):
    import ml_dtypes
    pv_np = ml_dtypes.bfloat16 if pv_bf16 else np.float32
    ident = np.eye(128, dtype=np.float32)
    maskbd = np.kron(np.eye(8, dtype=np.float32),
                     np.ones((16, 64), np.float32)).astype(pv_np)
    ones_row = np.ones((1, 128), np.float32)
    in_maps = []
    for core in range(8):
        b, hh = core // 2, core % 2
        cs = slice(hh * 256, (hh + 1) * 256)
        in_maps.append({
            "xl": np.ascontiguousarray(x_l[b]),
            "xs": np.ascontiguousarray(x_s[b]),
            "wq": np.ascontiguousarray(Wq[:, cs]),
            "wk": np.ascontiguousarray(Wk[:, cs]),
            "wv": np.ascontiguousarray(Wv[:, cs]),
            "bq2": np.ascontiguousarray(bq[cs].reshape(2, 128).T),
            "bk2": np.ascontiguousarray(bk[cs].reshape(2, 128).T),
            "bv_row": np.ascontiguousarray(bv[cs].reshape(1, 256)),
            "ident_f32": ident,
            "ident_pv": ident.astype(pv_np),
            "maskbd": maskbd,
            "ones_row": ones_row,
        })
    return in_maps


def _run(in_maps, pv_bf16=True, trace=False):
    from concourse.bass_utils import run_bass_kernel_spmd
    key = ("prog", pv_bf16)
    if key not in _CACHE:
        _CACHE[key] = _build_program(pv_bf16)
    nc = _CACHE[key]
    res = run_bass_kernel_spmd(nc, in_maps, core_ids=list(range(8)), trace=trace)
    return res


def kernel(x_s, x_l, Wq, bq, Wk, bk, Wv, bv, H=None, W=None, **_unused):
    x_s = np.asarray(x_s, np.float32)
    x_l = np.asarray(x_l, np.float32)
    in_maps = _host_inputs(np.asarray(x_l, np.float32), np.asarray(x_s, np.float32),
                           np.asarray(Wq, np.float32), np.asarray(bq, np.float32),
                           np.asarray(Wk, np.float32), np.asarray(bk, np.float32),
                           np.asarray(Wv, np.float32), np.asarray(bv, np.float32))
    res = _run(in_maps)
    out = np.empty((B, NH, NPATCH, N, HD), np.float32)
    for core in range(8):
        b, hh = core // 2, core % 2
        out[b, hh * 4:(hh + 1) * 4] = res.results[core]["out_c"]
    return out


# revision 5
# speedup vs baseline: 1.0786x; 1.0786x over previous
# Trainium2 Bass kernel for windowed cross-attention (nn_CrossAttention).
#
# Reference computation (shapes hardcoded):
#   B=4, N=1024 (=32x32), C=512, NH=8 heads, HD=64
#   q = x_l @ Wq + bq    -> [B, NH, N, HD]   (query patch = whole image)
#   k = x_s @ Wk + bk    -> [B, NH, N, HD]   patchified 4x4 -> 64 patches x 16 tok
#   v = x_s @ Wv + bv
#   out[b,h,P,l,:] = softmax(q[b,h,l,:] @ k_patch[b,h,P].T * HD**-0.5) @ v_patch[b,h,P]
#   out shape [4, 8, 64, 1024, 64] fp32  (512 MB -> memory-bound on output writes)
#
# Sharding: 8 cores = (B=4) x (head-half=2). Each core computes its b and 4 heads:
# per-core output [4, 64, 1024, 64] = 64 MB.
#
# Per-core dataflow (all on-chip after the input loads):
#   - load x_l[b], x_s[b]; transpose via PE to get C-major layouts; the x_s
#     transpose copies also permute tokens into patch-major order so K and V
#     come out patch-ordered for free.
#   - QT/KT = W.T @ x.T  (outC on partitions, tokens on free), fp32
#   - V = x.T.T @ Wv (tokens on partitions) with a ones-row matmul adding bv
#   - scores S = QT.T-slice @ KT (fp32), exp via ScalarE (scale folded in, no
#     max-subtraction: logits are O(1) by construction), segmented sums +
#     reciprocal + broadcast-normalize on VectorE
#   - P^T chunks via PE transpose; O = P^T.T @ blockdiag(V patches) where the
#     block-diagonal [128, 8*64] matrix zeroes cross-patch terms; this emits
#     outputs in natural [l, hd] layout, 8 patches per matmul.
#   - PSUM -> SBUF staging copies on ScalarE, 2 MB HWDGE DMAs to HBM.
#
# The P/V side of the second matmul runs in bf16 (fp32 accumulation in PSUM);
# projections, scores and softmax stay fp32. Measured vs float64 reference this
# gives ~4.6e-3 scale-relative absmax error (fp32 everywhere: ~7e-7, but fp32
# matmul is 4 cycles/row on the PE which would make the kernel PE-bound).

import numpy as np

B, N, C = 4, 1024, 512
NH, HD = 8, 64
H4 = 4          # heads per core
NPATCH = 64     # 4x4 key/value patches
PTOK = 16       # tokens per patch
SCALE = HD ** -0.5

_CACHE = {}


def _build_program(pv_bf16=True):
    import concourse.bass as bass
    import concourse.mybir as mybir
    from concourse import bacc
    from concourse.tile import TileContext

    f32 = mybir.dt.float32
    pv_dt = mybir.dt.bfloat16 if pv_bf16 else mybir.dt.float32
    X = mybir.AxisListType.X
    MULT = mybir.AluOpType.mult
    Exp = mybir.ActivationFunctionType.Exp
    Ident = mybir.ActivationFunctionType.Identity

    nc = bacc.Bacc("TRN2", target_bir_lowering=False, debug=False, num_devices=1)

    xl_d = nc.dram_tensor("xl", [N, C], f32, kind="ExternalInput")
    xs_d = nc.dram_tensor("xs", [N, C], f32, kind="ExternalInput")
    wq_d = nc.dram_tensor("wq", [C, 256], f32, kind="ExternalInput")
    wk_d = nc.dram_tensor("wk", [C, 256], f32, kind="ExternalInput")
    wv_d = nc.dram_tensor("wv", [C, 256], f32, kind="ExternalInput")
    bq_d = nc.dram_tensor("bq2", [128, 2], f32, kind="ExternalInput")
    bk_d = nc.dram_tensor("bk2", [128, 2], f32, kind="ExternalInput")
    bv_d = nc.dram_tensor("bv_row", [1, 256], f32, kind="ExternalInput")
    idf_d = nc.dram_tensor("ident_f32", [128, 128], f32, kind="ExternalInput")
    idp_d = nc.dram_tensor("ident_pv", [128, 128], pv_dt, kind="ExternalInput")
    msk_d = nc.dram_tensor("maskbd", [128, 512], pv_dt, kind="ExternalInput")
    one_d = nc.dram_tensor("ones_row", [1, 128], f32, kind="ExternalInput")
    out_d = nc.dram_tensor("out_c", [H4, NPATCH, N, HD], f32, kind="ExternalOutput")

    with TileContext(nc) as tc:
        with (
            tc.tile_pool(name="singles", bufs=1) as sg,
            tc.tile_pool(name="xnat", bufs=1) as xnat_p,
            tc.tile_pool(name="bdv", bufs=10) as bdv_p,
            tc.tile_pool(name="work", bufs=2) as wk_p,
            tc.tile_pool(name="pts", bufs=4) as pts_p,
            tc.tile_pool(name="small", bufs=4) as sm_p,
            tc.tile_pool(name="stage", bufs=2) as st_p,
            tc.tile_pool(name="psA", bufs=2, space="PSUM") as psA,
            tc.tile_pool(name="psB", bufs=2, space="PSUM") as psB,
            tc.tile_pool(name="psC", bufs=2, space="PSUM") as psC,
        ):
            # ---- constants ----
            wq = sg.tile([128, 4, 256], f32, name="wq_t")
            wk = sg.tile([128, 4, 256], f32, name="wk_t")
            wv = sg.tile([128, 4, 256], f32, name="wv_t")
            nc.sync.dma_start(wq[:], wq_d.ap().rearrange("(ko ki) m -> ki ko m", ki=128))
            nc.sync.dma_start(wk[:], wk_d.ap().rearrange("(ko ki) m -> ki ko m", ki=128))
            nc.sync.dma_start(wv[:], wv_d.ap().rearrange("(ko ki) m -> ki ko m", ki=128))
            bq2 = sg.tile([128, 2], f32, name="bq2_t")
            bk2 = sg.tile([128, 2], f32, name="bk2_t")
            bvr = sg.tile([1, 256], f32, name="bvr_t")
            nc.sync.dma_start(bq2[:], bq_d.ap())
            nc.sync.dma_start(bk2[:], bk_d.ap())
            nc.sync.dma_start(bvr[:], bv_d.ap())
            idf = sg.tile([128, 128], f32, name="idf_t")
            idp = sg.tile([128, 128], pv_dt, name="idp_t")
            msk = sg.tile([128, 512], pv_dt, name="msk_t")
            ones = sg.tile([1, 128], f32, name="ones_t")
            nc.sync.dma_start(idf[:], idf_d.ap())
            nc.sync.dma_start(idp[:], idp_d.ap())
            nc.sync.dma_start(msk[:], msk_d.ap())
            nc.sync.dma_start(ones[:], one_d.ap())

            # ---- persistent intermediates ----
            xlT = sg.tile([128, 4, N], f32, name="xlT")   # [c_lo, ko, token]
            xsT = sg.tile([128, 4, N], f32, name="xsT")   # tokens patch-permuted
            # Q/K kept in bf16 after exact fp32 projection: bf16 matmul runs the
            # PE at 1 cycle/row vs fp32's 4 (scores stay fp32 in PSUM).
            QT = sg.tile([128, 2, N], pv_dt, name="QT")   # [outC_lo, outC_tile, token]
            KT = sg.tile([128, 2, N], pv_dt, name="KT")   # tokens patch-permuted
            vperm = sg.tile([128, 8, 256], pv_dt, name="vperm")  # [tok(perm), tile, outC]

            # ---- x transposes (PE) ----
            # x_l: plain; x_s: token-permute into patch-major order during copy.
            for which, src_d, dstT, permute in ((0, xl_d, xlT, False), (1, xs_d, xsT, True)):
                xn = xnat_p.tile([128, 8, C], f32, tag="xnat")
                nc.sync.dma_start(xn[:], src_d.ap().rearrange("(tt p) c -> p tt c", p=128))
                for tt in range(8):
                    for ko in range(4):
                        tp = psB.tile([128, 128], f32, tag="pt_psum")
                        nc.tensor.transpose(tp[:], xn[:, tt, ko * 128:(ko + 1) * 128], idf[:])
                        if not permute:
                            nc.vector.tensor_copy(dstT[:, ko, tt * 128:(tt + 1) * 128], tp[:])
                        else:
                            blk = dstT[:, ko, tt * 128:(tt + 1) * 128]
                            blk3 = blk.rearrange("p (px k) -> p px k", px=8)
                            for dy in range(4):
                                src3 = tp[:, dy * 32:(dy + 1) * 32].rearrange(
                                    "p (px dx) -> p px dx", px=8)
                                nc.vector.tensor_copy(blk3[:, :, dy * 4:dy * 4 + 4], src3)

            # ---- projections ----
            # QT/KT: [outC, token] = W.T @ x.T ; bias per-partition via ScalarE.
            for wt, xt, dst, bias in ((wq, xlT, QT, bq2), (wk, xsT, KT, bk2)):
                for m in range(2):
                    pp = psC.tile([128, 1024], f32, tag="o_psum")
                    for n in range(2):
                        for ko in range(4):
                            nc.tensor.matmul(
                                pp[:, n * 512:(n + 1) * 512],
                                wt[:, ko, m * 128:(m + 1) * 128],
                                xt[:, ko, n * 512:(n + 1) * 512],
                                start=(ko == 0), stop=(ko == 3))
                    nc.scalar.activation(dst[:, m, :], pp[:], Ident,
                                         bias=bias[:, m:m + 1], scale=1.0)
            # V: [token(perm), outC] = x.T.T @ Wv + ones.T @ bv_row
            for tt in range(8):
                vp = psC.tile([128, 1024], f32, tag="o_psum")
                for ko in range(4):
                    nc.tensor.matmul(vp[:, :256], xsT[:, ko, tt * 128:(tt + 1) * 128],
                                     wv[:, ko, :], start=(ko == 0), stop=False)
                nc.tensor.matmul(vp[:, :256], ones[:, :],
                                 bvr[:], start=False, stop=True)
                nc.vector.tensor_copy(vperm[:, tt, :], vp[:, :256])

            # ---- blockdiag(V) per (head, group): bd[r, px*64+hd] =
            #      (r//16 == px) * V_perm[r, g, h*64+hd]  via mask multiply ----
            bd = {}
            for h in range(H4):
                for g in range(8):
                    t = bdv_p.tile([128, 512], pv_dt, tag="bdv")
                    nc.vector.tensor_tensor(
                        t.rearrange("p (px hd) -> p px hd", px=8),
                        msk.rearrange("p (px hd) -> p px hd", px=8),
                        vperm[:, g, h * 64:(h + 1) * 64][:, None, :].to_broadcast(
                            (128, 8, 64)),
                        MULT)
                    bd[(h, g)] = t

            # ---- main attention loop ----
            for h in range(H4):
                th, po = h // 2, (h % 2) * 64
                for qt in range(8):
                    E = wk_p.tile([128, 1024], f32, tag="E")
                    for n in range(2):
                        # S PSUM split per 512-half so the next iteration's
                        # scores can start as soon as each exp drains its half.
                        sp = psA.tile([128, 512], f32, tag="s_psum")
                        nc.tensor.matmul(
                            sp[:],
                            QT[po:po + 64, th, qt * 128:(qt + 1) * 128],
                            KT[po:po + 64, th, n * 512:(n + 1) * 512],
                            start=True, stop=True)
                        nc.scalar.activation(E[:, n * 512:(n + 1) * 512], sp[:],
                                             Exp, scale=SCALE)
                    sums = sm_p.tile([128, 64], f32, tag="sums")
                    nc.vector.reduce_sum(sums[:], E.rearrange("p (g s) -> p g s", s=16),
                                         axis=X)
                    rcp = sm_p.tile([128, 64], f32, tag="rcp")
                    nc.vector.reciprocal_approx_fast(rcp[:], sums[:])
                    Pn = wk_p.tile([128, 1024], pv_dt, tag="Pn")
                    nc.vector.tensor_tensor(
                        Pn.rearrange("p (g s) -> p g s", s=16),
                        E.rearrange("p (g s) -> p g s", s=16),
                        rcp[:, :, None].to_broadcast((128, 64, 16)),
                        MULT)
                    stage = st_p.tile([128, 4096], f32, tag="stage")
                    for gp in range(4):
                        op = psC.tile([128, 1024], f32, tag="o_psum")
                        for j in range(2):
                            g = gp * 2 + j
                            ptp = psB.tile([128, 128], pv_dt, tag="pt_psum")
                            nc.tensor.transpose(ptp[:], Pn[:, g * 128:(g + 1) * 128],
                                                idp[:])
                            pts = pts_p.tile([128, 128], pv_dt, tag="pts")
                            nc.vector.tensor_copy(pts[:], ptp[:])
                            nc.tensor.matmul(op[:, j * 512:(j + 1) * 512], pts[:],
                                             bd[(h, g)], start=True, stop=True)
                        nc.scalar.copy(stage[:, gp * 1024:(gp + 1) * 1024], op[:])
                    dst = out_d.ap()[h][:, qt * 128:(qt + 1) * 128, :].rearrange(
                        "P l hd -> l P hd")
                    nc.sync.dma_start(dst, stage.rearrange("p (P hd) -> p P hd", hd=64))

    nc.compile()
    return nc


def _host_inputs(x_l, x_s, Wq, bq, Wk, bk, Wv, bv, pv_bf16=True):
    import ml_dtypes
    pv_np = ml_dtypes.bfloat16 if pv_bf16 else np.float32
    ident = np.eye(128, dtype=np.float32)
    maskbd = np.kron(np.eye(8, dtype=np.float32),
                     np.ones((16, 64), np.float32)).astype(pv_np)
    ones_row = np.ones((1, 128), np.float32)
    in_maps = []
    for core in range(8):
        b, hh = core // 2, core % 2
        cs = slice(hh * 256, (hh + 1) * 256)
        in_maps.append({
            "xl": np.ascontiguousarray(x_l[b]),
            "xs": np.ascontiguousarray(x_s[b]),
            "wq": np.ascontiguousarray(Wq[:, cs]),
            "wk": np.ascontiguousarray(Wk[:, cs]),
            "wv": np.ascontiguousarray(Wv[:, cs]),
            "bq2": np.ascontiguousarray(bq[cs].reshape(2, 128).T),
            "bk2": np.ascontiguousarray(bk[cs].reshape(2, 128).T),
            "bv_row": np.ascontiguousarray(bv[cs].reshape(1, 256)),
            "ident_f32": ident,
            "ident_pv": ident.astype(pv_np),
            "maskbd": maskbd,
            "ones_row": ones_row,
        })
    return in_maps


def _run(in_maps, pv_bf16=True, trace=False):
    from concourse.bass_utils import run_bass_kernel_spmd
    key = ("prog", pv_bf16)
    if key not in _CACHE:
        _CACHE[key] = _build_program(pv_bf16)
    nc = _CACHE[key]
    res = run_bass_kernel_spmd(nc, in_maps, core_ids=list(range(8)), trace=trace)
    return res


def kernel(x_s, x_l, Wq, bq, Wk, bk, Wv, bv, H=None, W=None, **_unused):
    x_s = np.asarray(x_s, np.float32)
    x_l = np.asarray(x_l, np.float32)
    in_maps = _host_inputs(np.asarray(x_l, np.float32), np.asarray(x_s, np.float32),
                           np.asarray(Wq, np.float32), np.asarray(bq, np.float32),
                           np.asarray(Wk, np.float32), np.asarray(bk, np.float32),
                           np.asarray(Wv, np.float32), np.asarray(bv, np.float32))
    res = _run(in_maps)
    out = np.empty((B, NH, NPATCH, N, HD), np.float32)
    for core in range(8):
        b, hh = core // 2, core % 2
        out[b, hh * 4:(hh + 1) * 4] = res.results[core]["out_c"]
    return out


# revision 6
# speedup vs baseline: 1.2872x; 1.1934x over previous
# Trainium2 Bass kernel for windowed cross-attention (nn_CrossAttention).
#
# Reference computation (shapes hardcoded):
#   B=4, N=1024 (=32x32), C=512, NH=8 heads, HD=64
#   q = x_l @ Wq + bq    -> [B, NH, N, HD]   (query patch = whole image)
#   k = x_s @ Wk + bk    -> [B, NH, N, HD]   patchified 4x4 -> 64 patches x 16 tok
#   v = x_s @ Wv + bv
#   out[b,h,P,l,:] = softmax(q[b,h,l,:] @ k_patch[b,h,P].T * HD**-0.5) @ v_patch[b,h,P]
#   out shape [4, 8, 64, 1024, 64] fp32  (512 MB -> memory-bound on output writes)
#
# Sharding: 8 cores = (B=4) x (head-half=2). Each core computes its b and 4
# heads: per-core output [4, 64, 1024, 64] = 64 MB.
#
# Per-core dataflow:
#   - host pre-casts x/W to fp16 and pre-permutes x_s tokens into patch-major
#     order, so K and V come out patch-ordered and the xbar DMA transpose
#     (2-byte dtypes only) gives the C-major layouts with no PE transposes.
#   - QT/KT = W.T @ x.T (outC on partitions, fp16), V = x.T.T @ Wv with a
#     ones-row matmul adding bv (tokens on partitions, fp16)
#   - scores S = QT.T-slice @ KT (fp16 in, fp32 PSUM), exp on ScalarE (scale
#     folded, no max-subtraction: logits are O(1) by construction), segmented
#     sums + fast reciprocal + broadcast-normalize on VectorE -> P fp16
#   - P^T chunks via PE transpose; O = P^T.T @ blockdiag(V patches): the
#     block-diagonal [128, 8*64] fp16 matrix zeroes cross-patch terms and
#     emits outputs in natural [l, hd] layout, 8 patches per matmul.
#   - PSUM -> SBUF staging copies on ScalarE, 2 MB HWDGE DMAs to HBM.
#
# fp16 (not bf16): same 1 cycle/row PE speed, 4x finer mantissa. All values
# here are O(1) so fp16 range is a non-issue. Measured ~1e-3 scale-relative
# absmax error vs float64 (fp32 everywhere: ~7e-7 but 4 cycles/row on PE).

import numpy as np

B, N, C = 4, 1024, 512
NH, HD = 8, 64
H4 = 4          # heads per core
NPATCH = 64     # 4x4 key/value patches
PTOK = 16       # tokens per patch
SCALE = HD ** -0.5

_CACHE = {}


def _token_perm():
    # natural token t = (4*tt+dy)*32 + 4*px + dx  ->  patch-major position
    # tt*128 + px*16 + dy*4 + dx   (patch P = tt*8+px, within-patch k = dy*4+dx)
    perm = np.empty(N, np.int64)
    for tt in range(8):
        for px in range(8):
            for dy in range(4):
                for dx in range(4):
                    perm[tt * 128 + px * 16 + dy * 4 + dx] = \
                        (4 * tt + dy) * 32 + 4 * px + dx
    return perm


def _build_program():
    import concourse.bass as bass
    import concourse.mybir as mybir
    from concourse import bacc
    from concourse.tile import TileContext

    f32 = mybir.dt.float32
    lp = mybir.dt.float16
    X = mybir.AxisListType.X
    MULT = mybir.AluOpType.mult
    Exp = mybir.ActivationFunctionType.Exp
    Ident = mybir.ActivationFunctionType.Identity

    nc = bacc.Bacc("TRN2", target_bir_lowering=False, debug=False, num_devices=1)

    # x_s arrives token-permuted (patch-major) and fp16; x_l natural fp16.
    xl_d = nc.dram_tensor("xl", [N, C], lp, kind="ExternalInput")
    xs_d = nc.dram_tensor("xs", [N, C], lp, kind="ExternalInput")
    wq_d = nc.dram_tensor("wq", [C, 256], lp, kind="ExternalInput")
    wk_d = nc.dram_tensor("wk", [C, 256], lp, kind="ExternalInput")
    wv_d = nc.dram_tensor("wv", [C, 256], lp, kind="ExternalInput")
    bq_d = nc.dram_tensor("bq2", [128, 2], f32, kind="ExternalInput")
    bk_d = nc.dram_tensor("bk2", [128, 2], f32, kind="ExternalInput")
    bv_d = nc.dram_tensor("bv_row", [1, 256], lp, kind="ExternalInput")
    idp_d = nc.dram_tensor("ident_lp", [128, 128], lp, kind="ExternalInput")
    msk_d = nc.dram_tensor("maskbd", [128, 512], lp, kind="ExternalInput")
    one_d = nc.dram_tensor("ones_row", [1, 128], lp, kind="ExternalInput")
    out_d = nc.dram_tensor("out_c", [H4, NPATCH, N, HD], f32, kind="ExternalOutput")

    with TileContext(nc) as tc:
        with (
            tc.tile_pool(name="singles", bufs=1) as sg,
            tc.tile_pool(name="bdv", bufs=10) as bdv_p,
            tc.tile_pool(name="work", bufs=3) as wk_p,
            tc.tile_pool(name="pts", bufs=4) as pts_p,
            tc.tile_pool(name="small", bufs=4) as sm_p,
            tc.tile_pool(name="stage", bufs=3) as st_p,
            tc.tile_pool(name="psA", bufs=2, space="PSUM") as psA,
            tc.tile_pool(name="psB", bufs=2, space="PSUM") as psB,
            tc.tile_pool(name="psC", bufs=2, space="PSUM") as psC,
        ):
            # ---- constants / inputs (x_s path first: V+K gate first output) --
            xsT = sg.tile([128, 4, N], lp, name="xsT")   # [c_lo, ko, tok(perm)]
            for ko in range(4):
                nc.sync.dma_start(xsT[:, ko, :], xs_d.ap()[:, ko * 128:(ko + 1) * 128],
                                  transpose=True)
            wv = sg.tile([128, 4, 256], lp, name="wv_t")
            nc.sync.dma_start(wv[:], wv_d.ap().rearrange("(ko ki) m -> ki ko m", ki=128))
            wk = sg.tile([128, 4, 256], lp, name="wk_t")
            nc.sync.dma_start(wk[:], wk_d.ap().rearrange("(ko ki) m -> ki ko m", ki=128))
            bvr = sg.tile([1, 256], lp, name="bvr_t")
            nc.sync.dma_start(bvr[:], bv_d.ap())
            ones = sg.tile([1, 128], lp, name="ones_t")
            nc.sync.dma_start(ones[:], one_d.ap())
            msk = sg.tile([128, 512], lp, name="msk_t")
            nc.sync.dma_start(msk[:], msk_d.ap())
            idp = sg.tile([128, 128], lp, name="idp_t")
            nc.sync.dma_start(idp[:], idp_d.ap())
            xlT = sg.tile([128, 4, N], lp, name="xlT")   # [c_lo, ko, token]
            for ko in range(4):
                nc.sync.dma_start(xlT[:, ko, :], xl_d.ap()[:, ko * 128:(ko + 1) * 128],
                                  transpose=True)
            wq = sg.tile([128, 4, 256], lp, name="wq_t")
            nc.sync.dma_start(wq[:], wq_d.ap().rearrange("(ko ki) m -> ki ko m", ki=128))
            bq2 = sg.tile([128, 2], f32, name="bq2_t")
            bk2 = sg.tile([128, 2], f32, name="bk2_t")
            nc.sync.dma_start(bq2[:], bq_d.ap())
            nc.sync.dma_start(bk2[:], bk_d.ap())

            QT = sg.tile([128, 2, N], lp, name="QT")     # [outC_lo, tile, token]
            KT = sg.tile([128, 2, N], lp, name="KT")     # tokens patch-permuted
            vperm = sg.tile([128, 8, 256], lp, name="vperm")  # [tok(perm), tt, outC]

            # ---- V projection (tokens on partitions, patch order) ----
            for tt in range(8):
                vp = psC.tile([128, 1024], f32, tag="o_psum")
                for ko in range(4):
                    nc.tensor.matmul(vp[:, :256], xsT[:, ko, tt * 128:(tt + 1) * 128],
                                     wv[:, ko, :], start=(ko == 0), stop=False)
                nc.tensor.matmul(vp[:, :256], ones[:, :], bvr[:],
                                 start=False, stop=True)
                nc.vector.tensor_copy(vperm[:, tt, :], vp[:, :256])

            # ---- blockdiag(V) per (head, group): bd[r, px*64+hd] =
            #      (r//16 == px) * V_perm[r, g, h*64+hd]  via mask multiply ----
            bd = {}
            for h in range(H4):
                for g in range(8):
                    t = bdv_p.tile([128, 512], lp, tag="bdv")
                    nc.vector.tensor_tensor(
                        t.rearrange("p (px hd) -> p px hd", px=8),
                        msk.rearrange("p (px hd) -> p px hd", px=8),
                        vperm[:, g, h * 64:(h + 1) * 64][:, None, :].to_broadcast(
                            (128, 8, 64)),
                        MULT)
                    bd[(h, g)] = t

            # ---- K/Q projections: [outC, token] = W.T @ x.T, bias on ScalarE --
            for wt, xt, dst, bias in ((wk, xsT, KT, bk2), (wq, xlT, QT, bq2)):
                for m in range(2):
                    pp = psC.tile([128, 1024], f32, tag="o_psum")
                    for n in range(2):
                        for ko in range(4):
                            nc.tensor.matmul(
                                pp[:, n * 512:(n + 1) * 512],
                                wt[:, ko, m * 128:(m + 1) * 128],
                                xt[:, ko, n * 512:(n + 1) * 512],
                                start=(ko == 0), stop=(ko == 3))
                    nc.scalar.activation(dst[:, m, :], pp[:], Ident,
                                         bias=bias[:, m:m + 1], scale=1.0)

            # ---- main attention loop ----
            for h in range(H4):
                th, po = h // 2, (h % 2) * 64
                for qt in range(8):
                    E = wk_p.tile([128, 1024], f32, tag="E")
                    for n in range(2):
                        sp = psA.tile([128, 512], f32, tag="s_psum")
                        nc.tensor.matmul(
                            sp[:],
                            QT[po:po + 64, th, qt * 128:(qt + 1) * 128],
                            KT[po:po + 64, th, n * 512:(n + 1) * 512],
                            start=True, stop=True)
                        nc.scalar.activation(E[:, n * 512:(n + 1) * 512], sp[:],
                                             Exp, scale=SCALE)
                    sums = sm_p.tile([128, 64], f32, tag="sums")
                    nc.vector.reduce_sum(sums[:], E.rearrange("p (g s) -> p g s", s=16),
                                         axis=X)
                    rcp = sm_p.tile([128, 64], f32, tag="rcp")
                    nc.vector.reciprocal_approx_fast(rcp[:], sums[:])
                    Pn = wk_p.tile([128, 1024], lp, tag="Pn")
                    nc.vector.tensor_tensor(
                        Pn.rearrange("p (g s) -> p g s", s=16),
                        E.rearrange("p (g s) -> p g s", s=16),
                        rcp[:, :, None].to_broadcast((128, 64, 16)),
                        MULT)
                    stage = st_p.tile([128, 4096], f32, tag="stage")
                    for gp in range(4):
                        op = psC.tile([128, 1024], f32, tag="o_psum")
                        for j in range(2):
                            g = gp * 2 + j
                            ptp = psB.tile([128, 128], lp, tag="pt_psum")
                            nc.tensor.transpose(ptp[:], Pn[:, g * 128:(g + 1) * 128],
                                                idp[:])
                            pts = pts_p.tile([128, 128], lp, tag="pts")
                            nc.vector.tensor_copy(pts[:], ptp[:])
                            nc.tensor.matmul(op[:, j * 512:(j + 1) * 512], pts[:],
                                             bd[(h, g)], start=True, stop=True)
                        nc.scalar.copy(stage[:, gp * 1024:(gp + 1) * 1024], op[:])
                    dst = out_d.ap()[h][:, qt * 128:(qt + 1) * 128, :].rearrange(
                        "P l hd -> l P hd")
                    nc.sync.dma_start(dst, stage.rearrange("p (P hd) -> p P hd", hd=64))

    nc.compile()
    return nc


def _host_inputs(x_l, x_s, Wq, bq, Wk, bk, Wv, bv):
    f16 = np.float16
    perm = _token_perm()
    ident = np.eye(128, dtype=f16)
    maskbd = np.kron(np.eye(8, dtype=np.float32),
                     np.ones((16, 64), np.float32)).astype(f16)
    ones_row = np.ones((1, 128), f16)
    in_maps = []
    for core in range(8):
        b, hh = core // 2, core % 2
        cs = slice(hh * 256, (hh + 1) * 256)
        in_maps.append({
            "xl": np.ascontiguousarray(x_l[b].astype(f16)),
            "xs": np.ascontiguousarray(x_s[b][perm].astype(f16)),
            "wq": np.ascontiguousarray(Wq[:, cs].astype(f16)),
            "wk": np.ascontiguousarray(Wk[:, cs].astype(f16)),
            "wv": np.ascontiguousarray(Wv[:, cs].astype(f16)),
            "bq2": np.ascontiguousarray(bq[cs].reshape(2, 128).T.astype(np.float32)),
            "bk2": np.ascontiguousarray(bk[cs].reshape(2, 128).T.astype(np.float32)),
            "bv_row": bv[cs].reshape(1, 256).astype(f16),
            "ident_lp": ident,
            "maskbd": maskbd,
            "ones_row": ones_row,
        })
    return in_maps


def _run(in_maps, trace=False):
    from concourse.bass_utils import run_bass_kernel_spmd
    if "prog" not in _CACHE:
        _CACHE["prog"] = _build_program()
    nc = _CACHE["prog"]
    res = run_bass_kernel_spmd(nc, in_maps, core_ids=list(range(8)), trace=trace)
    return res


def kernel(x_s, x_l, Wq, bq, Wk, bk, Wv, bv, H=None, W=None, **_unused):
    in_maps = _host_inputs(np.asarray(x_l, np.float32), np.asarray(x_s, np.float32),
                           np.asarray(Wq, np.float32), np.asarray(bq, np.float32),
                           np.asarray(Wk, np.float32), np.asarray(bk, np.float32),
                           np.asarray(Wv, np.float32), np.asarray(bv, np.float32))
    res = _run(in_maps)
    out = np.empty((B, NH, NPATCH, N, HD), np.float32)
    for core in range(8):
        b, hh = core // 2, core % 2
        out[b, hh * 4:(hh + 1) * 4] = res.results[core]["out_c"]
    return out
